# revision 16
# baseline (speedup 1.0000x reference)
"""Trainium2 Bass kernel for nn_AdaptiveLinearWithChannel.

out[b,c,n,:] = x[b,c,n,:] @ weight[indices[c]] + bias[c,0,:] + hyper(t[b], c)
with hyper = per-channel relu MLP (1 -> 64 -> 64 -> 32) / DIN.

Sharding: channel dim split across 8 NeuronCores (16 channels each,
expert-parallel). Host-side packing (part of sharding) gathers the per-channel
weights, casts x to bf16 and lays it out transposed per 4-channel group:
xT[b, g, 32*c+i, n] = x[b, 4g+c, n, i]. All FLOPs over x and the hyper MLP run
on device.

Per-core device pipeline (bf16 in/out, f32 accumulate):
  - xT slab DMA (HWDGE, [128, slab] bf16, 16KB/partition contiguous lines)
  - matmul vs block-diagonal 4-channel weight (K = 4*32 = 128, stationary)
    producing outT[(c,o), n] directly -- no transposes anywhere on device
  - DVE per-partition scalar add of shift (bias + hyper MLP output), with
    f32 -> bf16 cast on write
  - bf16 DMA out (transposed layout; host un-transposes)

The hyper MLP runs fully transposed (features on partitions): layer 1 is an
outer product W1 x t, layers 2/3 are channel-pair block-diagonal matmuls, so
there are no PE transposes and the serial prologue chain is only ~6 deep.

HBM traffic per core: 32 MiB in + 32 MiB out (the f32 baseline moved 128 MiB).
"""

import sys

for _p in ("/opt/trn_rl_repo", "/opt/pypackages"):
    if _p not in sys.path:
        sys.path.append(_p)

import numpy as np
import ml_dtypes

import concourse.bass as bass
import concourse.mybir as mybir
from concourse import bacc
import concourse.tile as tile

B, C, N, DIN, DOUT, HID = 2, 128, 16384, 32, 32, 64
NCORES = 8
CS = C // NCORES          # channels per core = 16
G = CS // 4               # channel groups of 4 (partitions = 4*32 = 128)
NPAIR = CS // 2           # hyper block-diag pairs = 8

F32 = mybir.dt.float32
BF16 = mybir.dt.bfloat16
BF16_NP = ml_dtypes.bfloat16


def build_nc(n_points=N, reps=1, slab_pts=8192, xs_bufs=4, os_bufs=3,
             mm_bufs=3, mm_cols=1024, main_mode="full", pro_mode="mlp",
             out_eng="scalar", in_eng="sync"):
    """Build the per-core Bass graph. Same SPMD graph for all 8 cores.

    main_mode/pro_mode are timing-diagnostic ablations ("dma"/"nodve"/
    "dmain"/"dmaout" bypass stages); production is ("full", "mlp").
    out_eng picks the HWDGE issue ring for output DMAs ("sync" = SP,
    "scalar" = ACT) -- separate rings avoid head-of-line blocking of
    input DMAs behind semaphore-waiting output DMAs.
    """
    if n_points < slab_pts:
        slab_pts = n_points
    assert n_points % slab_pts == 0
    n_slabs = n_points // slab_pts
    NJ = slab_pts // mm_cols     # psum tiles per slab
    NM = mm_cols // 512          # matmuls per psum tile

    nc = bacc.Bacc("TRN2", target_bir_lowering=False, debug=False)

    xT = nc.dram_tensor("xT", [B, G, 128, n_points], BF16,
                        kind="ExternalInput")
    outT = nc.dram_tensor("outT", [B, G, 128, n_points], BF16,
                          kind="ExternalOutput")
    tT = nc.dram_tensor("tT", [1, B], F32, kind="ExternalInput")
    wblk = nc.dram_tensor("wblk", [128, G * 128], BF16, kind="ExternalInput")
    wh1 = nc.dram_tensor("wh1", [1, CS * HID], F32, kind="ExternalInput")
    wh2 = nc.dram_tensor("wh2", [128, NPAIR * 128], F32, kind="ExternalInput")
    wh3 = nc.dram_tensor("wh3", [128, NPAIR * 2 * DOUT], F32,
                         kind="ExternalInput")
    # biases / const shift, pre-transposed on host to (feature-partition, j*B)
    b1t = nc.dram_tensor("b1t", [128, NPAIR * B], F32, kind="ExternalInput")
    b2t = nc.dram_tensor("b2t", [128, NPAIR * B], F32, kind="ExternalInput")
    sct = nc.dram_tensor("sct", [128, G * B], F32, kind="ExternalInput")

    with tile.TileContext(nc) as tc:

        def body():
            with (
                tc.tile_pool(name="const", bufs=1) as const,
                tc.tile_pool(name="xs", bufs=xs_bufs) as xpool,
                tc.tile_pool(name="os", bufs=os_bufs) as opool,
                tc.tile_pool(name="mm", bufs=mm_bufs, space="PSUM") as mmpool,
                tc.tile_pool(name="hyp", bufs=1, space="PSUM") as hyppool,
            ):
                # ---- load constants (one DMA each) ----
                tT_t = const.tile([1, B], F32)
                nc.sync.dma_start(tT_t[:], tT[:])
                wblk_t = const.tile([128, G * 128], BF16)
                nc.sync.dma_start(wblk_t[:], wblk[:])
                wh1_t = const.tile([1, CS * HID], F32)
                nc.sync.dma_start(wh1_t[:], wh1[:])
                wh2_t = const.tile([128, NPAIR * 128], F32)
                nc.sync.dma_start(wh2_t[:], wh2[:])
                wh3_t = const.tile([128, NPAIR * 2 * DOUT], F32)
                nc.sync.dma_start(wh3_t[:], wh3[:])
                b1t_t = const.tile([128, NPAIR * B], F32)
                nc.sync.dma_start(b1t_t[:], b1t[:])
                b2t_t = const.tile([128, NPAIR * B], F32)
                nc.sync.dma_start(b2t_t[:], b2t[:])
                sct_t = const.tile([128, G * B], F32)
                nc.sync.dma_start(sct_t[:], sct[:])

                # ---- main loop: outT[(c,o), n] = wblk_g.T @ xT + shift ----
                oeng = nc.scalar if out_eng == "scalar" else nc.sync

                def _ieng(k):
                    if in_eng == "mixed":
                        return nc.gpsimd if k % 2 else nc.sync
                    return nc.sync

                def _oeng(k):
                    if out_eng == "mixed":
                        return nc.gpsimd if k % 2 else nc.scalar
                    return oeng

                def _main(shiftT):
                    if main_mode == "dmaout":
                        zs = const.tile([128, slab_pts], BF16)
                        nc.vector.memset(zs[:], 0.0)
                    k = 0
                    for b in range(B):
                        for g in range(G):
                            for s in range(n_slabs):
                                k += 1
                                n0 = s * slab_pts
                                if main_mode == "dmaout":
                                    _oeng(k).dma_start(
                                        outT[b, g, :, n0:n0 + slab_pts],
                                        zs[:])
                                    continue
                                xs = xpool.tile([128, slab_pts], BF16)
                                _ieng(k).dma_start(
                                    xs[:], xT[b, g, :, n0:n0 + slab_pts])
                                if main_mode == "dmain":
                                    continue
                                if main_mode == "dma":
                                    _oeng(k).dma_start(
                                        outT[b, g, :, n0:n0 + slab_pts],
                                        xs[:])
                                    continue
                                os_ = opool.tile([128, slab_pts], BF16)
                                for j in range(NJ):
                                    mm = mmpool.tile([128, mm_cols], F32,
                                                     tag="mm")
                                    for q in range(NM):
                                        sl = slice(j * mm_cols + q * 512,
                                                   j * mm_cols + (q + 1) * 512)
                                        nc.tensor.matmul(
                                            mm[:, q * 512:(q + 1) * 512],
                                            wblk_t[:, g * 128:(g + 1) * 128],
                                            xs[:, sl], start=True, stop=True)
                                    if main_mode == "nodve":
                                        continue
                                    osl = slice(j * mm_cols,
                                                (j + 1) * mm_cols)
                                    nc.vector.tensor_scalar_add(
                                        os_[:, osl], mm[:],
                                        shiftT[:, g * B + b:g * B + b + 1])
                                if main_mode == "nodve":
                                    _oeng(k).dma_start(
                                        outT[b, g, :, n0:n0 + slab_pts],
                                        xs[:])
                                else:
                                    _oeng(k).dma_start(
                                        outT[b, g, :, n0:n0 + slab_pts],
                                        os_[:])

                if pro_mode == "dma":
                    # diagnostic: skip the hyper MLP; shiftT := sct (approx)
                    shiftT = const.tile([128, G * B], F32)
                    nc.vector.tensor_copy(shiftT[:], sct_t[:])
                    return _main(shiftT)

                # ---- hyper MLP, fully transposed (features on partitions) --
                # h1T[(cpair,h), j*B+b] = W1[(c,h)] * t[b]   (outer product)
                h1_ps = hyppool.tile([128, NPAIR * B], F32, tag="hyp")
                for j in range(NPAIR):
                    nc.tensor.matmul(h1_ps[:, j * B:(j + 1) * B],
                                     wh1_t[0:1, j * 128:(j + 1) * 128],
                                     tT_t[:], start=True, stop=True)
                h1_sb = const.tile([128, NPAIR * B], F32)
                nc.vector.tensor_add(h1_sb[:], h1_ps[:], b1t_t[:])
                nc.vector.tensor_scalar_max(h1_sb[:], h1_sb[:], 0.0)

                h2_ps = hyppool.tile([128, NPAIR * B], F32, tag="hyp")
                for j in range(NPAIR):
                    nc.tensor.matmul(h2_ps[:, j * B:(j + 1) * B],
                                     wh2_t[:, j * 128:(j + 1) * 128],
                                     h1_sb[:, j * B:(j + 1) * B],
                                     start=True, stop=True)
                h2_sb = const.tile([128, NPAIR * B], F32)
                nc.vector.tensor_add(h2_sb[:], h2_ps[:], b2t_t[:])
                nc.vector.tensor_scalar_max(h2_sb[:], h2_sb[:], 0.0)

                # h3: pairs land on partition halves; j=2g -> 0:64 of group g
                h3_ps = hyppool.tile([128, G * B], F32, tag="hyp")
                for j in range(NPAIR):
                    g, half = j // 2, (j % 2) * 64
                    nc.tensor.matmul(h3_ps[half:half + 64, g * B:(g + 1) * B],
                                     wh3_t[:, j * 2 * DOUT:(j + 1) * 2 * DOUT],
                                     h2_sb[:, j * B:(j + 1) * B],
                                     start=True, stop=True)
                # shiftT[(c,o), g*B+b] = h3T/DIN + (biasT + hb3T/DIN)
                shiftT = const.tile([128, G * B], F32)
                nc.vector.scalar_tensor_tensor(
                    shiftT[:], h3_ps[:], 1.0 / DIN, sct_t[:],
                    op0=mybir.AluOpType.mult, op1=mybir.AluOpType.add)

                _main(shiftT)

        if reps == 1:
            body()
        else:
            with tc.For_i(0, reps, 1):
                body()

    nc.compile()
    return nc


def host_pack(x, indices, t, weight, bias, hW1, hb1, hW2, hb2, hW3, hb3,
              n_points=N):
    """Gather per-core channel shards + pack device input tensors."""
    idx = np.asarray(indices).astype(np.int64)
    xb = np.asarray(x, dtype=np.float32).astype(BF16_NP)
    in_maps = []
    for m in range(NCORES):
        c0 = m * CS
        ci = idx[c0:c0 + CS]
        wg = np.asarray(weight, np.float32)[ci]            # (CS,32,32)
        # NOTE: reference adds bias positionally (no indices gather)
        biasg = np.asarray(bias, np.float32)[c0:c0 + CS, 0, :]  # (CS,32)
        h1w = np.asarray(hW1, np.float32)[ci][:, 0, :]     # (CS,64)
        h1b = np.asarray(hb1, np.float32)[ci]              # (CS,64)
        h2w = np.asarray(hW2, np.float32)[ci]              # (CS,64,64)
        h2b = np.asarray(hb2, np.float32)[ci]              # (CS,64)
        h3w = np.asarray(hW3, np.float32)[ci]              # (CS,64,32)
        h3b = np.asarray(hb3, np.float32)[ci]              # (CS,32)

        # xT[b, g, 32*c+i, n] = x[b, c0+4g+c, n, i]
        xc = xb[:, c0:c0 + CS, :n_points, :]               # (B,CS,n,32)
        xTc = np.ascontiguousarray(xc.transpose(0, 1, 3, 2)) \
            .reshape(B, G, 128, n_points)

        wblk = np.zeros((128, G * 128), np.float32)
        for g in range(G):
            for c in range(4):
                wblk[32 * c:32 * c + 32,
                     g * 128 + 32 * c:g * 128 + 32 * c + 32] = wg[4 * g + c]
        wh2 = np.zeros((128, NPAIR * 128), np.float32)
        wh3 = np.zeros((128, NPAIR * 2 * DOUT), np.float32)
        for j in range(NPAIR):
            wh2[0:64, j * 128:j * 128 + 64] = h2w[2 * j]
            wh2[64:128, j * 128 + 64:j * 128 + 128] = h2w[2 * j + 1]
            wh3[0:64, j * 2 * DOUT:j * 2 * DOUT + DOUT] = h3w[2 * j]
            wh3[64:128, j * 2 * DOUT + DOUT:(j + 1) * 2 * DOUT] = \
                h3w[2 * j + 1]

        # per-pair bias columns, repeated for each b:
        # b1t[(cpair,h), j*B+b] = h1b[2j + cpair, h]
        b1t = np.repeat(h1b.reshape(NPAIR, 128).T[:, :, None], B,
                        axis=2).reshape(128, NPAIR * B)
        b2t = np.repeat(h2b.reshape(NPAIR, 128).T[:, :, None], B,
                        axis=2).reshape(128, NPAIR * B)
        sc = (biasg + h3b / DIN).reshape(G, 128).T         # (128, G)
        sct = np.repeat(sc[:, :, None], B, axis=2).reshape(128, G * B)

        in_maps.append({
            "xT": xTc,
            "tT": np.ascontiguousarray(np.asarray(t, np.float32).T),
            "wblk": wblk.astype(BF16_NP),
            "wh1": h1w.reshape(1, -1).astype(np.float32),
            "wh2": wh2,
            "wh3": wh3,
            "b1t": np.ascontiguousarray(b1t, dtype=np.float32),
            "b2t": np.ascontiguousarray(b2t, dtype=np.float32),
            "sct": np.ascontiguousarray(sct, dtype=np.float32),
        })
    return in_maps


_NC_CACHE = {}


def _get_nc(n_points=N, reps=1):
    key = (n_points, reps)
    if key not in _NC_CACHE:
        _NC_CACHE[key] = build_nc(n_points, reps)
    return _NC_CACHE[key]


def kernel(**inputs):
    import time
    from concourse.bass_utils import run_bass_kernel_spmd
    nc = _get_nc()
    in_maps = host_pack(**inputs)
    last_err = None
    for attempt in range(3):
        try:
            res = run_bass_kernel_spmd(nc, in_maps,
                                       core_ids=list(range(NCORES)))
            outs = []
            for m in range(NCORES):
                oT = np.asarray(res.results[m]["outT"])    # (B,G,128,N) bf16
                o = oT.reshape(B, G, 4, DOUT, N).transpose(0, 1, 2, 4, 3) \
                    .reshape(B, CS, N, DOUT)
                outs.append(o)
            return np.concatenate(outs, axis=1).astype(np.float32)
        except Exception as e:  # transient NRT_EXEC_UNIT_UNRECOVERABLE etc.
            last_err = e
            time.sleep(20)
    raise last_err


if __name__ == "__main__":
    nc = build_nc()
    n = sum(len(bb.instructions) for bb in nc.main_func.blocks)
    print(f"built ok: {n} instructions")
